# revision 15
# baseline (speedup 1.0000x reference)
"""GQA attention (B=2, S=2048, H=2048, 16 Q heads / 4 KV heads, d=128) on
8 TRN2 NeuronCores.

Sharding: core c = (batch b = c//4, kv-group g = c%4). Each core computes
Q/K/V projections and attention for its 4 Q heads of its batch, then four
8-wide AllToAlls (one per local head, issued as soon as that head's
attention finishes so comm overlaps compute) redistribute attention
outputs head-sharded -> sequence-sharded; sends are duplicated to both
batch halves and a per-core mask selects the right half on receive.
Finally each core computes the full-width o_proj for its sequence quarter
in two passes (heads 0-2, then head 3) so the last A2A hides behind
~50us of PE work.

Changes vs the earlier 382us version:
- hidden^T is pre-transposed on the HOST, eliminating 256 PE transposes
  and their PSUM->SBUF copies from phase 1 (fp8 variants were tried and
  rejected: the attention output is a ~1/sqrt(n)-scale average, so input
  quantization error passes through at full relative strength; fp8's
  ~3-5% can never meet the 2e-2 gate).
- exp runs on 2+1-ktile score groups (a [128,1024] ACTIVATE then a
  [128,512] one), amortizing the ACT engine's ~352-cycle per-instruction
  overhead and keeping the exp stream under the PE stream.
- The transpose-flush PSUM tile shares the scB slot (frees a bank).
- o_proj output is DMA'd per 512-column block as it completes.

All matmuls run in bf16 with f32 PSUM accumulation; softmax runs without
max-subtraction (scores are O(5)) with the denominator computed for free
via a ones-column appended to V.
"""
import math
import sys
import types

import ml_dtypes
import numpy as np

if "/opt/trn_rl_repo" not in sys.path:
    sys.path.insert(0, "/opt/trn_rl_repo")


def _install_ntff_hook():
    """Register the axon NTFF profile hook (missing antenv.axon_hooks shim)."""
    if "antenv.axon_hooks" in sys.modules:
        return
    mod = types.ModuleType("antenv.axon_hooks")
    _h = [None]
    mod.set_axon_ntff_profile_hook = lambda h: _h.__setitem__(0, h)
    mod.get_axon_ntff_profile_hook = lambda: _h[0]
    sys.modules["antenv.axon_hooks"] = mod
    try:
        import antenv
        antenv.axon_hooks = mod
        from trn_agent_boot.trn_boot import _ntff_profile_via_ctypes
        mod.set_axon_ntff_profile_hook(
            _ntff_profile_via_ctypes("/opt/axon/libaxon_pjrt.so")
        )
    except Exception:
        pass


_install_ntff_hook()

import concourse.bass_utils as _bass_utils
_bass_utils.upload_artifacts = lambda d: d  # no artifact bucket in this env

import concourse.bacc as bacc
import concourse.tile as tile
import concourse.mybir as mybir
from concourse.bass_utils import run_bass_kernel_spmd

BF16 = mybir.dt.bfloat16
F32 = mybir.dt.float32

B, S, H = 2, 2048, 2048
D = 128              # head dim
NHL = 4              # local Q heads per core
NT = 16              # 128-tiles along H / S / attn-dim
NQC = 4              # 512-wide q chunks
QC = 512
N_CORES = 8
SCALE = 1.0 / math.sqrt(D)

_CACHE = {}


def _build():
    if "nc" in _CACHE:
        return _CACHE["nc"]

    nc = bacc.Bacc("TRN2", target_bir_lowering=False, debug=False,
                   num_devices=N_CORES)

    # hidden^T tiled: hidT[t, p, s] = hidden[s, t*128+p]
    hid_ext = nc.dram_tensor("hidT", [NT, 128, S], BF16, kind="ExternalInput")
    wq_ext = nc.dram_tensor("wq", [NT, 128, NHL * D], BF16,
                            kind="ExternalInput")
    wk_ext = nc.dram_tensor("wk", [NT, 128, D], BF16, kind="ExternalInput")
    wv_ext = nc.dram_tensor("wv", [NT, 128, D], BF16, kind="ExternalInput")
    wo_ext = nc.dram_tensor("wo", [H, H], BF16, kind="ExternalInput")
    id_ext = nc.dram_tensor("ident", [128, 128], BF16, kind="ExternalInput")
    bm_ext = nc.dram_tensor("bmask", [128, 2], F32, kind="ExternalInput")
    out_ext = nc.dram_tensor("out", [QC, H], F32, kind="ExternalOutput")

    with tile.TileContext(nc) as tc:
        with tc.tile_pool(name="dram", bufs=1, space="DRAM") as dram, \
             tc.tile_pool(name="persist", bufs=1) as per, \
             tc.tile_pool(name="attpool", bufs=4) as atp, \
             tc.tile_pool(name="work", bufs=3) as wk_pool, \
             tc.tile_pool(name="scpoolA", bufs=1, space="PSUM") as scA, \
             tc.tile_pool(name="scpoolB", bufs=1, space="PSUM") as scB, \
             tc.tile_pool(name="accpool", bufs=4, space="PSUM") as psacc, \
             tc.tile_pool(name="qpool", bufs=1, space="PSUM") as psq_p:

        # PSUM budget: scA 1x[128,1024]=2 banks, scB 1x[128,512]=1,
        # acc 4x[128,129]=4 (one bank each: a start=True matmul clears
        # has_written for its WHOLE bank, so accumulation groups must
        # never share one), psq 1 -> 8 banks.  The transpose-flush tile
        # borrows the scB slot.  Scores batch 2+1 kt per exp (scA/scB),
        # keeping the ACT stream (~1.9us/3kt) under the PE stream
        # (~2.4us/3kt) while both sc banks ping-pong stall-free.

            ident = per.tile([128, 128], BF16, name="ident_sb")
            nc.sync.dma_start(ident[:], id_ext[:])
            bmask = per.tile([128, 2], F32, name="bmask_sb")
            nc.sync.dma_start(bmask[:], bm_ext[:])

            qT = [per.tile([128, S], BF16, name=f"qT{h}") for h in range(NHL)]
            kT = per.tile([128, S], BF16, name="kT")
            v_aug = [per.tile([128, D + 1], BF16, name=f"vaug{st}")
                     for st in range(NT)]
            # per-head A2A bounce buffers
            send = [dram.tile([N_CORES, 128, QC], BF16, name=f"send{h}")
                    for h in range(NHL)]
            recv = [dram.tile([N_CORES, 128, QC], BF16, name=f"recv{h}")
                    for h in range(NHL)]
            gathered = [per.tile([128, QC], BF16, name=f"gat{at}")
                        for at in range(NT)]
            rtiles = {}

            last_copy = [None]

            def attention(h, proj=None):
                # The transpose+copy+send of a q-chunk only depends on its
                # normalized ob tiles, so defer it by one chunk: the next
                # chunk's scores matmul then issues immediately after the
                # last PV instead of waiting behind the PE transposes.
                #
                # `proj`, if given, is (next_head, wq_sb, hidT): one qT
                # projection matmul is emitted per kt, so the projection
                # hides inside the attention stream instead of
                # serializing as a block at the head boundary.
                #
                # PV+qproj for score-group m are emitted AFTER the scores
                # of group m+1 (one-group software pipeline), so the
                # single-buffered score banks never stall the PE: exp(m)
                # runs while the PE does PV(m-1)+qproj+scores(m+1).
                pending = [None]

                def flush(qc, obs):
                    tp = scB.tile([128, QC], F32, tag="scB",
                                  name=f"tpo_{h}_{qc}")
                    for qs in range(4):
                        nc.tensor.matmul(tp[:, qs * 128:(qs + 1) * 128],
                                         lhsT=obs[qs][:], rhs=ident[:],
                                         start=True, stop=True)
                    at_h = atp.tile([128, QC], BF16, tag="attnT",
                                    name=f"attnT_{h}_{qc}")
                    last_copy[0] = nc.vector.tensor_copy(at_h[:], tp[:])
                    # A2A sends for this q-chunk (dest rank qc of both halves)
                    nc.sync.dma_start(send[h][qc], at_h[:])
                    nc.sync.dma_start(send[h][4 + qc], at_h[:])

                for qc in range(NQC):
                    cs = slice(qc * QC, (qc + 1) * QC)
                    acc = [psacc.tile([128, D + 1], F32, tag="acc",
                                      name=f"acc_{h}_{qc}_{qs}")[:]
                           for qs in range(4)]
                    if proj is not None:
                        ph, wq_sb, hidT = proj
                        psq = psq_p.tile([128, QC], F32, tag="psq",
                                         name=f"psq_{ph}_{qc}")
                    prev = [None]

                    def pv_group(m, pts):
                        for kt, pt, off in pts:
                            for qs in range(4):
                                nc.tensor.matmul(
                                    acc[qs],
                                    lhsT=pt[:, off + qs * 128:
                                            off + (qs + 1) * 128],
                                    rhs=v_aug[kt][:],
                                    start=(kt == 0), stop=(kt == NT - 1))
                            if proj is not None:
                                nc.tensor.matmul(
                                    psq[:],
                                    lhsT=wq_sb[kt][:, ph * D:(ph + 1) * D],
                                    rhs=hidT[kt][:, cs],
                                    start=(kt == 0), stop=(kt == NT - 1))

                    # 6 score groups per q-chunk: m<5 -> kt (3m, 3m+1)
                    # batched in scA + kt 3m+2 in scB; m=5 -> kt 15 in scB
                    for m in range(6):
                        pts = []
                        if m < 5:
                            sc2 = scA.tile([128, 2 * QC], F32, tag="scA",
                                           name=f"scA_{h}_{qc}_{m}")
                            for half in range(2):
                                kt = 3 * m + half
                                nc.tensor.matmul(
                                    sc2[:, half * QC:(half + 1) * QC],
                                    lhsT=kT[:, kt * 128:(kt + 1) * 128],
                                    rhs=qT[h][:, cs], start=True, stop=True)
                            pt2 = wk_pool.tile([128, 2 * QC], BF16,
                                               tag="pt",
                                               name=f"ptA_{h}_{qc}_{m}")
                            nc.scalar.activation(
                                pt2[:], sc2[:],
                                mybir.ActivationFunctionType.Exp,
                                scale=SCALE)
                            pts.append((3 * m, pt2, 0))
                            pts.append((3 * m + 1, pt2, QC))
                        ktb = 3 * m + 2 if m < 5 else 15
                        sc1 = scB.tile([128, QC], F32, tag="scB",
                                       name=f"scB_{h}_{qc}_{m}")
                        nc.tensor.matmul(
                            sc1[:], lhsT=kT[:, ktb * 128:(ktb + 1) * 128],
                            rhs=qT[h][:, cs], start=True, stop=True)
                        pt1 = wk_pool.tile([128, QC], BF16, tag="pt",
                                           name=f"ptB_{h}_{qc}_{m}")
                        nc.scalar.activation(
                            pt1[:], sc1[:],
                            mybir.ActivationFunctionType.Exp, scale=SCALE)
                        pts.append((ktb, pt1, 0))
                        if prev[0] is not None:
                            pv_group(*prev[0])
                        prev[0] = (m, pts)
                        if m == 1 and pending[0] is not None:
                            flush(*pending[0])
                            pending[0] = None
                    pv_group(*prev[0])
                    # normalize (frees the acc banks for the next chunk)
                    obs = []
                    for qs in range(4):
                        rec = wk_pool.tile([128, 1], F32, tag="rec",
                                           name=f"rec_{h}_{qc}_{qs}")
                        nc.vector.reciprocal(rec[:], acc[qs][:, D:])
                        ob = wk_pool.tile([128, D], BF16, tag="ob", bufs=8,
                                          name=f"ob_{h}_{qc}_{qs}")
                        nc.vector.tensor_scalar_mul(ob[:], acc[qs][:, :D],
                                                    rec[:])
                        obs.append(ob)
                    if proj is not None:
                        nc.vector.tensor_copy(qT[ph][:, cs], psq[:])
                    pending[0] = (qc, obs)
                flush(*pending[0])
                # A2A for this head, overlapped with the next head's compute
                nc.gpsimd.collective_compute(
                    "AllToAll", mybir.AluOpType.bypass,
                    replica_groups=[list(range(N_CORES))],
                    ins=[send[h][:]], outs=[recv[h][:]],
                )

            def recv_load(h, eng=None):
                # recv DMAs for head h, emitted right after the NEXT head's
                # collective issue so they sit early in the gpsimd stream
                # (head h's collective has finished by then -> no stall)
                for gp in range(4):
                    lo = wk_pool.tile([128, QC], BF16, tag="rlo", bufs=12,
                                      name=f"rlo_{h}_{gp}")
                    hi = wk_pool.tile([128, QC], BF16, tag="rhi", bufs=12,
                                      name=f"rhi_{h}_{gp}")
                    e = eng if eng is not None else nc.gpsimd
                    e.dma_start(out=lo[:], in_=recv[h][gp])
                    e.dma_start(out=hi[:], in_=recv[h][4 + gp])
                    rtiles[(h, gp)] = (lo, hi)

            def combine(h, after=None):
                # receive-side batch mask:
                # gathered[4*gp + h] = recv_lo*m0 + recv_hi*m1
                # Explicitly ordered after the last attention's DVE work
                # (or, for the last head, after o_proj pass-1's copies) so
                # the collective wait can never stall the DVE stream:
                # Tile's cost model underestimates the collective and
                # would otherwise hoist these ahead of copies the PE's
                # PSUM-slot rotation depends on.
                order_after = after if after is not None else last_copy[0]
                for gp in range(4):
                    lo, hi = rtiles[(h, gp)]
                    mul = nc.vector.tensor_scalar_mul(hi[:], hi[:],
                                                      bmask[:, 1:2])
                    if order_after is not None:
                        tile.add_dep_helper(
                            mul.ins, order_after.ins, sync=False,
                            reason="combine ordered behind critical copies")
                    nc.vector.scalar_tensor_tensor(
                        gathered[4 * gp + h][:], lo[:], bmask[:, 0:1], hi[:],
                        mybir.AluOpType.mult, mybir.AluOpType.add)

            with tc.tile_pool(name="projpool", bufs=1) as pp:

                hidT = [pp.tile([128, S], BF16, name=f"hidT{t}")
                        for t in range(NT)]
                wq_sb = [pp.tile([128, NHL * D], BF16, name=f"wq{t}")
                         for t in range(NT)]
                wk_sb = [pp.tile([128, D], BF16, name=f"wk{t}")
                         for t in range(NT)]
                wv_sb = [pp.tile([128, D], BF16, name=f"wv{t}")
                         for t in range(NT)]

                # wk on the scalar ring (needed first); wv/wq on the
                # otherwise-idle gpsimd ring
                for t in range(NT):
                    nc.scalar.dma_start(out=wk_sb[t][:], in_=wk_ext[t])
                for t in range(NT):
                    nc.gpsimd.dma_start(out=wv_sb[t][:], in_=wv_ext[t])
                for t in range(NT):
                    nc.gpsimd.dma_start(out=wq_sb[t][:], in_=wq_ext[t])

                # ---- phase 1: stream hidT in 512-column chunks; as each
                # chunk lands, run the kT / v / q0 projections for it.
                for c in range(4):
                    cs = slice(c * QC, (c + 1) * QC)
                    for t in range(NT):
                        nc.sync.dma_start(out=hidT[t][:, cs],
                                          in_=hid_ext[t][:, cs])
                    # kT for this s-chunk
                    psk = psq_p.tile([128, QC], F32, tag="psq",
                                     name=f"psk_{c}")
                    for t in range(NT):
                        nc.tensor.matmul(psk[:], lhsT=wk_sb[t][:],
                                         rhs=hidT[t][:, cs],
                                         start=(t == 0), stop=(t == NT - 1))
                    nc.vector.tensor_copy(kT[:, cs], psk[:])
                    # v tiles for this s-chunk
                    psv = scB.tile([128, QC], F32, tag="scB",
                                   name=f"psv_{c}")
                    for sl in range(4):
                        st = 4 * c + sl
                        for t in range(NT):
                            nc.tensor.matmul(
                                psv[:, sl * 128:(sl + 1) * 128],
                                lhsT=hidT[t][:, st * 128:(st + 1) * 128],
                                rhs=wv_sb[t][:],
                                start=(t == 0), stop=(t == NT - 1))
                    for sl in range(4):
                        st = 4 * c + sl
                        nc.vector.tensor_copy(
                            v_aug[st][:, :D], psv[:, sl * 128:(sl + 1) * 128])
                        nc.vector.memset(v_aug[st][:, D:], 1.0)
                    # q0 for this s-chunk
                    ps0 = scA.tile([128, QC], F32, tag="scA",
                                   name=f"psq0_{c}")
                    for t in range(NT):
                        nc.tensor.matmul(ps0[:], lhsT=wq_sb[t][:, 0:D],
                                         rhs=hidT[t][:, cs],
                                         start=(t == 0), stop=(t == NT - 1))
                    nc.vector.tensor_copy(qT[0][:, cs], ps0[:])

                # ---- phase 3: attention; heads 1-3 project one matmul
                # per kt inside the previous head's attention
                for h in range(NHL - 1):
                    attention(h, proj=(h + 1, wq_sb, hidT))
                    if h >= 1:
                        recv_load(h - 1)

            # projpool closed: hidT/wq freed; wo loads reuse that space and
            # overlap the last head's attention.
            with tc.tile_pool(name="late", bufs=1) as lp:
                wo_sb = [lp.tile([128, H], BF16, name=f"wo{at}")
                         for at in range(NT)]
                # wo on the gpsimd ring: the sync ring must stay free for
                # the last head's sends (a queued wo load would delay its
                # collective)
                for at in range(NT):
                    nc.gpsimd.dma_start(
                        out=wo_sb[at][:], in_=wo_ext[at * 128:(at + 1) * 128, :])

                attention(NHL - 1)
                recv_load(NHL - 2)
                recv_load(NHL - 1, eng=nc.sync)
                for h in range(NHL - 1):
                    combine(h)

                # ---- phase 5: o_proj for my seq quarter ----
                # pass 1 accumulates heads 0-2 for ALL 16 output groups
                # (~50us of PE work with no h3 dependency -> the last A2A
                # is fully hidden even on slow-fabric runs), staging
                # partials in f32 SBUF rows; pass 2 adds head 3's 4-step
                # psum and streams each 512-col block out.  combine(3) is
                # emitted between the passes so its collective wait can
                # never block pass-1's PSUM-evacuation copies; the copies
                # alternate DVE/ACT so neither queue serializes.
                ats1 = [4 * gp + h for h in range(NHL - 1) for gp in range(4)]
                ats2 = [4 * gp + (NHL - 1) for gp in range(4)]

                def oslot(tag8, g):
                    if g < 2:
                        if oslot.sca is None:
                            oslot.sca = scA.tile([128, 2 * QC], F32,
                                                 tag="scA",
                                                 name=f"psoA_{tag8}")
                        return oslot.sca[:, g * QC:(g + 1) * QC]
                    if g == 2:
                        return scB.tile([128, QC], F32, tag="scB",
                                        name=f"psoB_{tag8}")[:]
                    if g == 3:
                        return psq_p.tile([128, QC], F32, tag="psq",
                                          name=f"psoQ_{tag8}")[:]
                    return psacc.tile([128, QC], F32, tag="acc",
                                      name=f"psoC_{tag8}_{g}")[:]

                groups = [(st, hc) for st in range(4) for hc in range(4)]
                orows = [wk_pool.tile([128, H], F32, tag="orow", bufs=4,
                                      name=f"orow{st}") for st in range(4)]
                p1_last = None
                for gi, (st, hc) in enumerate(groups):
                    if gi % 8 == 0:
                        oslot.sca = None
                    ps = oslot(f"p1_{gi // 8}", gi % 8)
                    for i, at in enumerate(ats1):
                        nc.tensor.matmul(
                            ps, lhsT=gathered[at][:, st * 128:
                                                  (st + 1) * 128],
                            rhs=wo_sb[at][:, hc * QC:(hc + 1) * QC],
                            start=(i == 0), stop=(i == len(ats1) - 1))
                    dst = orows[st][:, hc * QC:(hc + 1) * QC]
                    if gi % 2 == 0:
                        p1_last = nc.vector.tensor_copy(dst, ps)
                    else:
                        nc.scalar.copy(dst, ps)
                combine(NHL - 1, after=p1_last)
                for gi, (st, hc) in enumerate(groups):
                    if gi % 8 == 0:
                        oslot.sca = None
                    ps = oslot(f"p2_{gi // 8}", gi % 8)
                    for i, at in enumerate(ats2):
                        nc.tensor.matmul(
                            ps, lhsT=gathered[at][:, st * 128:
                                                  (st + 1) * 128],
                            rhs=wo_sb[at][:, hc * QC:(hc + 1) * QC],
                            start=(i == 0), stop=(i == len(ats2) - 1))
                    blk = slice(hc * QC, (hc + 1) * QC)
                    nc.vector.tensor_tensor(
                        orows[st][:, blk], orows[st][:, blk], ps,
                        mybir.AluOpType.add)
                    oeng = nc.sync if gi % 2 == 0 else nc.gpsimd
                    oeng.dma_start(
                        out_ext[st * 128:(st + 1) * 128, blk],
                        orows[st][:, blk])

    nc.compile()
    _CACHE["nc"] = nc
    return nc


def _make_in_maps(hidden_states, w_q, w_k, w_v, w_o):
    bf16 = ml_dtypes.bfloat16
    ident = np.eye(128, dtype=bf16)
    hidT = [np.ascontiguousarray(hidden_states[b].T.astype(bf16)
                                 .reshape(NT, 128, S))
            for b in range(B)]
    wq_t = np.ascontiguousarray(w_q.astype(bf16).reshape(NT, 128, -1))
    wk_t = np.ascontiguousarray(w_k.astype(bf16).reshape(NT, 128, -1))
    wv_t = np.ascontiguousarray(w_v.astype(bf16).reshape(NT, 128, -1))
    wo_bf = np.ascontiguousarray(w_o.astype(bf16))
    in_maps = []
    for c in range(N_CORES):
        b, g = c // 4, c % 4
        m0 = 1.0 if b == 0 else 0.0
        bmask = np.empty((128, 2), np.float32)
        bmask[:, 0] = m0
        bmask[:, 1] = 1.0 - m0
        in_maps.append({
            "hidT": hidT[b],
            "wq": np.ascontiguousarray(wq_t[:, :, g * NHL * D:(g + 1) * NHL * D]),
            "wk": np.ascontiguousarray(wk_t[:, :, g * D:(g + 1) * D]),
            "wv": np.ascontiguousarray(wv_t[:, :, g * D:(g + 1) * D]),
            "wo": wo_bf,
            "ident": ident,
            "bmask": bmask,
        })
    return in_maps


def _run(hidden_states, w_q, w_k, w_v, w_o, trace=False):
    nc = _build()
    in_maps = _make_in_maps(hidden_states, w_q, w_k, w_v, w_o)
    res = run_bass_kernel_spmd(nc, in_maps, list(range(N_CORES)), trace=trace)
    out = np.empty((B, S, H), np.float32)
    for c in range(N_CORES):
        b, q = c // 4, c % 4
        out[b, q * QC:(q + 1) * QC, :] = res.results[c]["out"]
    return out, res


def kernel(hidden_states, position_ids=None, w_q=None, w_k=None, w_v=None,
           w_o=None):
    hidden_states = np.asarray(hidden_states, dtype=np.float32)
    w_q = np.asarray(w_q, dtype=np.float32)
    w_k = np.asarray(w_k, dtype=np.float32)
    w_v = np.asarray(w_v, dtype=np.float32)
    w_o = np.asarray(w_o, dtype=np.float32)
    out, _ = _run(hidden_states, w_q, w_k, w_v, w_o, trace=False)
    return out


# revision 16
# speedup vs baseline: 1.1369x; 1.1369x over previous
"""GQA attention (B=2, S=2048, H=2048, 16 Q heads / 4 KV heads, d=128) on
8 TRN2 NeuronCores.

Sharding: core c = (batch b = c//4, kv-group g = c%4). Each core computes
Q/K/V projections and attention for its 4 Q heads of its batch, then four
8-wide AllToAlls (one per local head, issued as soon as that head's
attention finishes so comm overlaps compute) redistribute attention
outputs head-sharded -> sequence-sharded; sends are duplicated to both
batch halves and a per-core mask selects the right half on receive.
Finally each core computes the full-width o_proj for its sequence quarter
in two passes (heads 0-2, then head 3) so the last A2A hides behind
~50us of PE work.

Changes vs the earlier 382us version:
- hidden^T is pre-transposed on the HOST, eliminating 256 PE transposes
  and their PSUM->SBUF copies from phase 1 (fp8 variants were tried and
  rejected: the attention output is a ~1/sqrt(n)-scale average, so input
  quantization error passes through at full relative strength; fp8's
  ~3-5% can never meet the 2e-2 gate).
- exp runs on 2+1-ktile score groups (a [128,1024] ACTIVATE then a
  [128,512] one), amortizing the ACT engine's ~352-cycle per-instruction
  overhead and keeping the exp stream under the PE stream.
- The transpose-flush PSUM tile shares the scB slot (frees a bank).
- o_proj output is DMA'd per 512-column block as it completes.

All matmuls run in bf16 with f32 PSUM accumulation; softmax runs without
max-subtraction (scores are O(5)) with the denominator computed for free
via a ones-column appended to V.
"""
import math
import sys
import types

import ml_dtypes
import numpy as np

if "/opt/trn_rl_repo" not in sys.path:
    sys.path.insert(0, "/opt/trn_rl_repo")


def _install_ntff_hook():
    """Register the axon NTFF profile hook (missing antenv.axon_hooks shim)."""
    if "antenv.axon_hooks" in sys.modules:
        return
    mod = types.ModuleType("antenv.axon_hooks")
    _h = [None]
    mod.set_axon_ntff_profile_hook = lambda h: _h.__setitem__(0, h)
    mod.get_axon_ntff_profile_hook = lambda: _h[0]
    sys.modules["antenv.axon_hooks"] = mod
    try:
        import antenv
        antenv.axon_hooks = mod
        from trn_agent_boot.trn_boot import _ntff_profile_via_ctypes
        mod.set_axon_ntff_profile_hook(
            _ntff_profile_via_ctypes("/opt/axon/libaxon_pjrt.so")
        )
    except Exception:
        pass


_install_ntff_hook()

import concourse.bass_utils as _bass_utils
_bass_utils.upload_artifacts = lambda d: d  # no artifact bucket in this env

import concourse.bacc as bacc
import concourse.tile as tile
import concourse.mybir as mybir
from concourse.bass_utils import run_bass_kernel_spmd

BF16 = mybir.dt.bfloat16
F32 = mybir.dt.float32

B, S, H = 2, 2048, 2048
D = 128              # head dim
NHL = 4              # local Q heads per core
NT = 16              # 128-tiles along H / S / attn-dim
NQC = 4              # 512-wide q chunks
QC = 512
N_CORES = 8
SCALE = 1.0 / math.sqrt(D)

_CACHE = {}


def _build():
    if "nc" in _CACHE:
        return _CACHE["nc"]

    nc = bacc.Bacc("TRN2", target_bir_lowering=False, debug=False,
                   num_devices=N_CORES)

    # hidden^T tiled: hidT[t, p, s] = hidden[s, t*128+p]
    hid_ext = nc.dram_tensor("hidT", [NT, 128, S], BF16, kind="ExternalInput")
    wq_ext = nc.dram_tensor("wq", [NT, 128, NHL * D], BF16,
                            kind="ExternalInput")
    wk_ext = nc.dram_tensor("wk", [NT, 128, D], BF16, kind="ExternalInput")
    wv_ext = nc.dram_tensor("wv", [NT, 128, D], BF16, kind="ExternalInput")
    wo_ext = nc.dram_tensor("wo", [H, H], BF16, kind="ExternalInput")
    id_ext = nc.dram_tensor("ident", [128, 128], BF16, kind="ExternalInput")
    bm_ext = nc.dram_tensor("bmask", [128, 2], F32, kind="ExternalInput")
    out_ext = nc.dram_tensor("out", [QC, H], F32, kind="ExternalOutput")

    with tile.TileContext(nc) as tc:
        with tc.tile_pool(name="dram", bufs=1, space="DRAM") as dram, \
             tc.tile_pool(name="persist", bufs=1) as per, \
             tc.tile_pool(name="attpool", bufs=4) as atp, \
             tc.tile_pool(name="work", bufs=3) as wk_pool, \
             tc.tile_pool(name="scpoolA", bufs=1, space="PSUM") as scA, \
             tc.tile_pool(name="scpoolB", bufs=1, space="PSUM") as scB, \
             tc.tile_pool(name="accpool", bufs=4, space="PSUM") as psacc, \
             tc.tile_pool(name="qpool", bufs=1, space="PSUM") as psq_p:

        # PSUM budget: scA 1x[128,1024]=2 banks, scB 1x[128,512]=1,
        # acc 4x[128,129]=4 (one bank each: a start=True matmul clears
        # has_written for its WHOLE bank, so accumulation groups must
        # never share one), psq 1 -> 8 banks.  The transpose-flush tile
        # borrows the scB slot.  Scores batch 2+1 kt per exp (scA/scB),
        # keeping the ACT stream (~1.9us/3kt) under the PE stream
        # (~2.4us/3kt) while both sc banks ping-pong stall-free.

            ident = per.tile([128, 128], BF16, name="ident_sb")
            nc.sync.dma_start(ident[:], id_ext[:])
            bmask = per.tile([128, 2], F32, name="bmask_sb")
            nc.sync.dma_start(bmask[:], bm_ext[:])

            qT = [per.tile([128, S], BF16, name=f"qT{h}") for h in range(NHL)]
            kT = per.tile([128, S], BF16, name="kT")
            v_aug = [per.tile([128, D + 1], BF16, name=f"vaug{st}")
                     for st in range(NT)]
            # per-head A2A bounce buffers
            send = [dram.tile([N_CORES, 128, QC], BF16, name=f"send{h}")
                    for h in range(NHL)]
            recv = [dram.tile([N_CORES, 128, QC], BF16, name=f"recv{h}")
                    for h in range(NHL)]
            gathered = [per.tile([128, QC], BF16, name=f"gat{at}")
                        for at in range(NT)]
            rtiles = {}

            last_copy = [None]

            def attention(h, proj=None):
                # The transpose+copy+send of a q-chunk only depends on its
                # normalized ob tiles, so defer it by one chunk: the next
                # chunk's scores matmul then issues immediately after the
                # last PV instead of waiting behind the PE transposes.
                #
                # `proj`, if given, is (next_head, wq_sb, hidT): one qT
                # projection matmul is emitted per kt, so the projection
                # hides inside the attention stream instead of
                # serializing as a block at the head boundary.
                #
                # PV+qproj for score-group m are emitted AFTER the scores
                # of group m+1 (one-group software pipeline), so the
                # single-buffered score banks never stall the PE: exp(m)
                # runs while the PE does PV(m-1)+qproj+scores(m+1).
                pending = [None]

                def flush(qc, obs):
                    tp = scB.tile([128, QC], F32, tag="scB",
                                  name=f"tpo_{h}_{qc}")
                    for qs in range(4):
                        nc.tensor.matmul(tp[:, qs * 128:(qs + 1) * 128],
                                         lhsT=obs[qs][:], rhs=ident[:],
                                         start=True, stop=True)
                    at_h = atp.tile([128, QC], BF16, tag="attnT",
                                    name=f"attnT_{h}_{qc}")
                    last_copy[0] = nc.vector.tensor_copy(at_h[:], tp[:])
                    # A2A sends for this q-chunk (dest rank qc of both halves)
                    nc.sync.dma_start(send[h][qc], at_h[:])
                    nc.sync.dma_start(send[h][4 + qc], at_h[:])

                for qc in range(NQC):
                    cs = slice(qc * QC, (qc + 1) * QC)
                    acc = [psacc.tile([128, D + 1], F32, tag="acc",
                                      name=f"acc_{h}_{qc}_{qs}")[:]
                           for qs in range(4)]
                    if proj is not None:
                        ph, wq_sb, hidT = proj
                        psq = psq_p.tile([128, QC], F32, tag="psq",
                                         name=f"psq_{ph}_{qc}")
                    prev = [None]

                    def pv_group(m, pts):
                        for kt, pt, off in pts:
                            for qs in range(4):
                                nc.tensor.matmul(
                                    acc[qs],
                                    lhsT=pt[:, off + qs * 128:
                                            off + (qs + 1) * 128],
                                    rhs=v_aug[kt][:],
                                    start=(kt == 0), stop=(kt == NT - 1))
                            if proj is not None:
                                nc.tensor.matmul(
                                    psq[:],
                                    lhsT=wq_sb[kt][:, ph * D:(ph + 1) * D],
                                    rhs=hidT[kt][:, cs],
                                    start=(kt == 0), stop=(kt == NT - 1))

                    # 6 score groups per q-chunk: m<5 -> kt (3m, 3m+1)
                    # batched in scA + kt 3m+2 in scB; m=5 -> kt 15 in scB
                    for m in range(6):
                        pts = []
                        if m < 5:
                            sc2 = scA.tile([128, 2 * QC], F32, tag="scA",
                                           name=f"scA_{h}_{qc}_{m}")
                            for half in range(2):
                                kt = 3 * m + half
                                nc.tensor.matmul(
                                    sc2[:, half * QC:(half + 1) * QC],
                                    lhsT=kT[:, kt * 128:(kt + 1) * 128],
                                    rhs=qT[h][:, cs], start=True, stop=True)
                            pt2 = wk_pool.tile([128, 2 * QC], BF16,
                                               tag="pt",
                                               name=f"ptA_{h}_{qc}_{m}")
                            nc.scalar.activation(
                                pt2[:], sc2[:],
                                mybir.ActivationFunctionType.Exp,
                                scale=SCALE)
                            pts.append((3 * m, pt2, 0))
                            pts.append((3 * m + 1, pt2, QC))
                        ktb = 3 * m + 2 if m < 5 else 15
                        sc1 = scB.tile([128, QC], F32, tag="scB",
                                       name=f"scB_{h}_{qc}_{m}")
                        nc.tensor.matmul(
                            sc1[:], lhsT=kT[:, ktb * 128:(ktb + 1) * 128],
                            rhs=qT[h][:, cs], start=True, stop=True)
                        pt1 = wk_pool.tile([128, QC], BF16, tag="pt",
                                           name=f"ptB_{h}_{qc}_{m}")
                        nc.scalar.activation(
                            pt1[:], sc1[:],
                            mybir.ActivationFunctionType.Exp, scale=SCALE)
                        pts.append((ktb, pt1, 0))
                        if prev[0] is not None:
                            pv_group(*prev[0])
                        prev[0] = (m, pts)
                        if m == 1 and pending[0] is not None:
                            flush(*pending[0])
                            pending[0] = None
                    pv_group(*prev[0])
                    # normalize (frees the acc banks for the next chunk)
                    obs = []
                    for qs in range(4):
                        rec = wk_pool.tile([128, 1], F32, tag="rec",
                                           name=f"rec_{h}_{qc}_{qs}")
                        nc.vector.reciprocal(rec[:], acc[qs][:, D:])
                        ob = wk_pool.tile([128, D], BF16, tag="ob", bufs=8,
                                          name=f"ob_{h}_{qc}_{qs}")
                        nc.vector.tensor_scalar_mul(ob[:], acc[qs][:, :D],
                                                    rec[:])
                        obs.append(ob)
                    if proj is not None:
                        nc.vector.tensor_copy(qT[ph][:, cs], psq[:])
                    pending[0] = (qc, obs)
                flush(*pending[0])
                # A2A for this head, overlapped with the next head's compute
                nc.gpsimd.collective_compute(
                    "AllToAll", mybir.AluOpType.bypass,
                    replica_groups=[list(range(N_CORES))],
                    ins=[send[h][:]], outs=[recv[h][:]],
                )

            def recv_load(h, eng=None):
                # recv DMAs for head h, emitted right after the NEXT head's
                # collective issue so they sit early in the gpsimd stream
                # (head h's collective has finished by then -> no stall)
                for gp in range(4):
                    lo = wk_pool.tile([128, QC], BF16, tag="rlo", bufs=12,
                                      name=f"rlo_{h}_{gp}")
                    hi = wk_pool.tile([128, QC], BF16, tag="rhi", bufs=12,
                                      name=f"rhi_{h}_{gp}")
                    e = eng if eng is not None else nc.gpsimd
                    e.dma_start(out=lo[:], in_=recv[h][gp])
                    e.dma_start(out=hi[:], in_=recv[h][4 + gp])
                    rtiles[(h, gp)] = (lo, hi)

            def combine(h, after=None):
                # receive-side batch mask:
                # gathered[4*gp + h] = recv_lo*m0 + recv_hi*m1
                # Explicitly ordered after the last attention's DVE work
                # (or, for the last head, after o_proj pass-1's copies) so
                # the collective wait can never stall the DVE stream:
                # Tile's cost model underestimates the collective and
                # would otherwise hoist these ahead of copies the PE's
                # PSUM-slot rotation depends on.
                order_after = after if after is not None else last_copy[0]
                for gp in range(4):
                    lo, hi = rtiles[(h, gp)]
                    mul = nc.vector.tensor_scalar_mul(hi[:], hi[:],
                                                      bmask[:, 1:2])
                    if order_after is not None:
                        tile.add_dep_helper(
                            mul.ins, order_after.ins, sync=False,
                            reason="combine ordered behind critical copies")
                    nc.vector.scalar_tensor_tensor(
                        gathered[4 * gp + h][:], lo[:], bmask[:, 0:1], hi[:],
                        mybir.AluOpType.mult, mybir.AluOpType.add)

            with tc.tile_pool(name="projpool", bufs=1) as pp:

                hidT = [pp.tile([128, S], BF16, name=f"hidT{t}")
                        for t in range(NT)]
                wq_sb = [pp.tile([128, NHL * D], BF16, name=f"wq{t}")
                         for t in range(NT)]
                wk_sb = [pp.tile([128, D], BF16, name=f"wk{t}")
                         for t in range(NT)]
                wv_sb = [pp.tile([128, D], BF16, name=f"wv{t}")
                         for t in range(NT)]

                # wk on the scalar ring (needed first); wv/wq on the
                # otherwise-idle gpsimd ring
                for t in range(NT):
                    nc.scalar.dma_start(out=wk_sb[t][:], in_=wk_ext[t])
                for t in range(NT):
                    nc.gpsimd.dma_start(out=wv_sb[t][:], in_=wv_ext[t])
                for t in range(NT):
                    nc.gpsimd.dma_start(out=wq_sb[t][:], in_=wq_ext[t])

                # ---- phase 1: stream hidT in 512-column chunks; as each
                # chunk lands, run the kT / v / q0 projections for it.
                for c in range(4):
                    cs = slice(c * QC, (c + 1) * QC)
                    for t in range(NT):
                        nc.sync.dma_start(out=hidT[t][:, cs],
                                          in_=hid_ext[t][:, cs])
                    # kT for this s-chunk
                    psk = psq_p.tile([128, QC], F32, tag="psq",
                                     name=f"psk_{c}")
                    for t in range(NT):
                        nc.tensor.matmul(psk[:], lhsT=wk_sb[t][:],
                                         rhs=hidT[t][:, cs],
                                         start=(t == 0), stop=(t == NT - 1))
                    nc.vector.tensor_copy(kT[:, cs], psk[:])
                    # v tiles for this s-chunk
                    psv = scB.tile([128, QC], F32, tag="scB",
                                   name=f"psv_{c}")
                    for sl in range(4):
                        st = 4 * c + sl
                        for t in range(NT):
                            nc.tensor.matmul(
                                psv[:, sl * 128:(sl + 1) * 128],
                                lhsT=hidT[t][:, st * 128:(st + 1) * 128],
                                rhs=wv_sb[t][:],
                                start=(t == 0), stop=(t == NT - 1))
                    for sl in range(4):
                        st = 4 * c + sl
                        nc.vector.tensor_copy(
                            v_aug[st][:, :D], psv[:, sl * 128:(sl + 1) * 128])
                        nc.vector.memset(v_aug[st][:, D:], 1.0)
                    # q0 for this s-chunk
                    ps0 = scA.tile([128, QC], F32, tag="scA",
                                   name=f"psq0_{c}")
                    for t in range(NT):
                        nc.tensor.matmul(ps0[:], lhsT=wq_sb[t][:, 0:D],
                                         rhs=hidT[t][:, cs],
                                         start=(t == 0), stop=(t == NT - 1))
                    nc.vector.tensor_copy(qT[0][:, cs], ps0[:])

                # ---- phase 3: attention; heads 1-3 project one matmul
                # per kt inside the previous head's attention
                for h in range(NHL - 1):
                    attention(h, proj=(h + 1, wq_sb, hidT))
                    if h >= 1:
                        recv_load(h - 1)

            # projpool closed: hidT/wq freed; wo loads reuse that space and
            # overlap the last head's attention.
            with tc.tile_pool(name="late", bufs=1) as lp:
                wo_sb = [lp.tile([128, H], BF16, name=f"wo{at}")
                         for at in range(NT)]
                # wo on the gpsimd ring: the sync ring must stay free for
                # the last head's sends (a queued wo load would delay its
                # collective)
                for at in range(NT):
                    nc.gpsimd.dma_start(
                        out=wo_sb[at][:], in_=wo_ext[at * 128:(at + 1) * 128, :])

                attention(NHL - 1)
                recv_load(NHL - 2)
                recv_load(NHL - 1, eng=nc.sync)
                for h in range(NHL - 1):
                    combine(h)

                # ---- phase 5: o_proj for my seq quarter ----
                # pass 1 accumulates heads 0-2 for ALL 16 output groups
                # (~50us of PE work with no h3 dependency -> the last A2A
                # is fully hidden even on slow-fabric runs), staging
                # partials in f32 SBUF rows; pass 2 adds head 3's 4-step
                # psum and streams each 512-col block out.  combine(3) is
                # emitted between the passes so its collective wait can
                # never block pass-1's PSUM-evacuation copies; the copies
                # alternate DVE/ACT so neither queue serializes.
                ats1 = [4 * gp + h for h in range(NHL - 1) for gp in range(4)]
                ats2 = [4 * gp + (NHL - 1) for gp in range(4)]

                def oslot(tag8, g):
                    if g < 2:
                        if oslot.sca is None:
                            oslot.sca = scA.tile([128, 2 * QC], F32,
                                                 tag="scA",
                                                 name=f"psoA_{tag8}")
                        return oslot.sca[:, g * QC:(g + 1) * QC]
                    if g == 2:
                        return scB.tile([128, QC], F32, tag="scB",
                                        name=f"psoB_{tag8}")[:]
                    if g == 3:
                        return psq_p.tile([128, QC], F32, tag="psq",
                                          name=f"psoQ_{tag8}")[:]
                    return psacc.tile([128, QC], F32, tag="acc",
                                      name=f"psoC_{tag8}_{g}")[:]

                groups = [(st, hc) for st in range(4) for hc in range(4)]
                orows = [wk_pool.tile([128, H], F32, tag="orow", bufs=4,
                                      name=f"orow{st}") for st in range(4)]
                # at-major accumulation within each batch of 8 groups:
                # all of head h's steps for the 8 groups run before head
                # h+1's tiles are touched, giving each head's A2A an
                # extra ~8us of slack on slow-fabric runs.
                p1_last = None
                for bat in range(2):
                    oslot.sca = None
                    bgroups = groups[bat * 8:(bat + 1) * 8]
                    slots = [oslot(f"p1_{bat}", g) for g in range(8)]
                    for i, at in enumerate(ats1):
                        for g, (st, hc) in enumerate(bgroups):
                            nc.tensor.matmul(
                                slots[g],
                                lhsT=gathered[at][:, st * 128:
                                                  (st + 1) * 128],
                                rhs=wo_sb[at][:, hc * QC:(hc + 1) * QC],
                                start=(i == 0), stop=(i == len(ats1) - 1))
                    for g, (st, hc) in enumerate(bgroups):
                        dst = orows[st][:, hc * QC:(hc + 1) * QC]
                        if g % 2 == 0:
                            p1_last = nc.vector.tensor_copy(dst, slots[g])
                        else:
                            nc.scalar.copy(dst, slots[g])
                combine(NHL - 1, after=p1_last)
                for gi, (st, hc) in enumerate(groups):
                    if gi % 8 == 0:
                        oslot.sca = None
                    ps = oslot(f"p2_{gi // 8}", gi % 8)
                    for i, at in enumerate(ats2):
                        nc.tensor.matmul(
                            ps, lhsT=gathered[at][:, st * 128:
                                                  (st + 1) * 128],
                            rhs=wo_sb[at][:, hc * QC:(hc + 1) * QC],
                            start=(i == 0), stop=(i == len(ats2) - 1))
                    blk = slice(hc * QC, (hc + 1) * QC)
                    nc.vector.tensor_tensor(
                        orows[st][:, blk], orows[st][:, blk], ps,
                        mybir.AluOpType.add)
                    oeng = nc.sync if gi % 2 == 0 else nc.gpsimd
                    oeng.dma_start(
                        out_ext[st * 128:(st + 1) * 128, blk],
                        orows[st][:, blk])

    nc.compile()
    _CACHE["nc"] = nc
    return nc


def _make_in_maps(hidden_states, w_q, w_k, w_v, w_o):
    bf16 = ml_dtypes.bfloat16
    ident = np.eye(128, dtype=bf16)
    hidT = [np.ascontiguousarray(hidden_states[b].T.astype(bf16)
                                 .reshape(NT, 128, S))
            for b in range(B)]
    wq_t = np.ascontiguousarray(w_q.astype(bf16).reshape(NT, 128, -1))
    wk_t = np.ascontiguousarray(w_k.astype(bf16).reshape(NT, 128, -1))
    wv_t = np.ascontiguousarray(w_v.astype(bf16).reshape(NT, 128, -1))
    wo_bf = np.ascontiguousarray(w_o.astype(bf16))
    in_maps = []
    for c in range(N_CORES):
        b, g = c // 4, c % 4
        m0 = 1.0 if b == 0 else 0.0
        bmask = np.empty((128, 2), np.float32)
        bmask[:, 0] = m0
        bmask[:, 1] = 1.0 - m0
        in_maps.append({
            "hidT": hidT[b],
            "wq": np.ascontiguousarray(wq_t[:, :, g * NHL * D:(g + 1) * NHL * D]),
            "wk": np.ascontiguousarray(wk_t[:, :, g * D:(g + 1) * D]),
            "wv": np.ascontiguousarray(wv_t[:, :, g * D:(g + 1) * D]),
            "wo": wo_bf,
            "ident": ident,
            "bmask": bmask,
        })
    return in_maps


def _run(hidden_states, w_q, w_k, w_v, w_o, trace=False):
    nc = _build()
    in_maps = _make_in_maps(hidden_states, w_q, w_k, w_v, w_o)
    res = run_bass_kernel_spmd(nc, in_maps, list(range(N_CORES)), trace=trace)
    out = np.empty((B, S, H), np.float32)
    for c in range(N_CORES):
        b, q = c // 4, c % 4
        out[b, q * QC:(q + 1) * QC, :] = res.results[c]["out"]
    return out, res


def kernel(hidden_states, position_ids=None, w_q=None, w_k=None, w_v=None,
           w_o=None):
    hidden_states = np.asarray(hidden_states, dtype=np.float32)
    w_q = np.asarray(w_q, dtype=np.float32)
    w_k = np.asarray(w_k, dtype=np.float32)
    w_v = np.asarray(w_v, dtype=np.float32)
    w_o = np.asarray(w_o, dtype=np.float32)
    out, _ = _run(hidden_states, w_q, w_k, w_v, w_o, trace=False)
    return out
